# revision 2
# baseline (speedup 1.0000x reference)
"""Trainium2 Bass kernel for nn_BlockDense_89730456748629.

Block-diagonal dense layer + ReLU:
    out[b, g*H+h] = relu( sum_w inputs[b, g*WIN+w] * W[g*WIN+w, g*H+h] )
with G=32 groups, WIN=128, H=256, B=4096.

Sharding: group-parallel over 8 NeuronCores — core c owns groups
[4c, 4c+4). Each core gets the matching 512 input columns of `inputs`
(pre-transposed on host so the contraction dim lies on SBUF partitions)
plus its 4 diagonal W blocks, and produces the matching 1024 output
columns. No cross-core communication.

Per-core device pipeline:
  DMA xT group-row (2MB) -> PE matmul (lhsT = xT tile [128win,128b],
  rhs = W block [128win,256h], PSUM fp32) -> ReLU fused into the
  PSUM->SBUF copy (alternating VectorE / ScalarE) -> 1MB batched DMA out.
"""

import os

import numpy as np

G, WIN, H, B = 32, 128, 256, 4096
NCORES = 8
GPC = G // NCORES            # groups per core
COLS_IN_PC = GPC * WIN       # 512 input columns per core
COLS_OUT_PC = GPC * H        # 1024 output columns per core
NB = B // 128                # 32 batch tiles of 128 rows
CH = 8                       # batch tiles per output DMA chunk (1MB fp32)

# dtype config: f32 | f32r | f16 | bf16 for inputs/matmul, f32 | f16 | bf16 out
IN_DT = os.environ.get("KERNEL_IN_DT", "f32")
OUT_DT = os.environ.get("KERNEL_OUT_DT", "f32")

LAST = None  # BassKernelResults of the most recent run (for test harness)

_cache = {}


def _np_dt(tag):
    if tag in ("f32", "f32r"):
        return np.float32
    if tag == "f16":
        return np.float16
    if tag == "bf16":
        import ml_dtypes

        return np.dtype(ml_dtypes.bfloat16)
    raise ValueError(tag)


def _mybir_dt(tag):
    from concourse import mybir

    return {
        "f32": mybir.dt.float32,
        "f32r": mybir.dt.float32r,
        "f16": mybir.dt.float16,
        "bf16": mybir.dt.bfloat16,
    }[tag]


def _build(in_tag, out_tag):
    from concourse import bacc, mybir, tile

    in_dt = _mybir_dt(in_tag)
    out_dt = _mybir_dt(out_tag)

    nc = bacc.Bacc(
        "TRN2", target_bir_lowering=False, debug=False, num_devices=NCORES
    )
    xT = nc.declare_dram_parameter("xT", [COLS_IN_PC, B], in_dt, isOutput=False)
    Wb = nc.declare_dram_parameter("Wb", [WIN, COLS_OUT_PC], in_dt, isOutput=False)
    out = nc.declare_dram_parameter("out", [B, COLS_OUT_PC], out_dt, isOutput=True)

    out_v = out.rearrange("(nb p) w -> nb p w", p=128)  # (NB, 128, COLS_OUT_PC)

    with tile.TileContext(nc) as tc:
        with (
            tc.tile_pool(name="w", bufs=1) as wpool,
            tc.tile_pool(name="x", bufs=2) as xpool,
            tc.tile_pool(name="o", bufs=4) as opool,
            tc.tile_pool(name="ps", bufs=8, space="PSUM") as pspool,
        ):
            wt = wpool.tile([WIN, COLS_OUT_PC], in_dt)
            nc.sync.dma_start(wt[:], Wb[:, :])
            for g in range(GPC):
                xt = xpool.tile([WIN, B], in_dt)
                nc.sync.dma_start(xt[:], xT[g * WIN : (g + 1) * WIN, :])
                for c in range(NB // CH):
                    ob = opool.tile([128, CH, H], out_dt)
                    for j in range(CH):
                        bt = c * CH + j
                        ps = pspool.tile([128, H], mybir.dt.float32)
                        nc.tensor.matmul(
                            ps[:],
                            xt[:, bt * 128 : (bt + 1) * 128],
                            wt[:, g * H : (g + 1) * H],
                            start=True,
                            stop=True,
                        )
                        if bt % 2 == 0:
                            nc.vector.tensor_scalar_max(ob[:, j, :], ps[:], 0.0)
                        else:
                            nc.scalar.activation(
                                ob[:, j, :],
                                ps[:],
                                mybir.ActivationFunctionType.Relu,
                            )
                    dv = out_v[
                        c * CH : (c + 1) * CH, :, g * H : (g + 1) * H
                    ].transpose([1, 0, 2])
                    nc.sync.dma_start(dv, ob[:])
    nc.compile()
    return nc


def kernel(inputs, W):
    global LAST
    from concourse.bass_utils import run_bass_kernel_spmd

    key = (IN_DT, OUT_DT)
    if key not in _cache:
        _cache[key] = _build(*key)
    nc = _cache[key]

    in_np = _np_dt(IN_DT)
    x = np.asarray(inputs, dtype=np.float32)
    Wf = np.asarray(W, dtype=np.float32)

    xT = np.ascontiguousarray(x.T)  # (G*WIN, B): row g*WIN+w = input column
    # Diagonal blocks of W: (G, WIN, H)
    Wd = Wf.reshape(G, WIN, G, H)[np.arange(G), :, np.arange(G), :]

    in_maps = []
    for c in range(NCORES):
        xTc = xT[c * COLS_IN_PC : (c + 1) * COLS_IN_PC, :].astype(in_np)
        Wc = (
            Wd[c * GPC : (c + 1) * GPC]
            .transpose(1, 0, 2)
            .reshape(WIN, COLS_OUT_PC)
            .astype(in_np)
        )
        in_maps.append(
            {"xT": np.ascontiguousarray(xTc), "Wb": np.ascontiguousarray(Wc)}
        )

    # NTFF tracing is unavailable on the bare axon image (antenv.axon_hooks
    # missing) — always run untraced; timing is measured by repeat-delta.
    LAST = run_bass_kernel_spmd(nc, in_maps, list(range(NCORES)), trace=False)

    outs = [
        np.asarray(LAST.results[c]["out"]).astype(np.float32)
        for c in range(NCORES)
    ]
    return np.concatenate(outs, axis=1)


# revision 3
# speedup vs baseline: 979.6439x; 979.6439x over previous
"""Trainium2 Bass kernel for nn_BlockDense_89730456748629.

Block-diagonal dense layer + ReLU:
    out[b, g*H+h] = relu( sum_w inputs[b, g*WIN+w] * W[g*WIN+w, g*H+h] )
with G=32 groups, WIN=128, H=256, B=4096.

Sharding: group-parallel over 8 NeuronCores — core c owns groups
[4c, 4c+4). Each core gets the matching 512 input columns of `inputs`
(pre-transposed on host so the contraction dim lies on SBUF partitions)
plus its 4 diagonal W blocks, and produces the matching 1024 output
columns. No cross-core communication.

Per-core device pipeline:
  DMA xT group-row (2MB) -> PE matmul (lhsT = xT tile [128win,128b],
  rhs = W block [128win,256h], PSUM fp32) -> ReLU fused into the
  PSUM->SBUF copy (alternating VectorE / ScalarE) -> 1MB batched DMA out.
"""

import os
import time

import numpy as np

G, WIN, H, B = 32, 128, 256, 4096
NCORES = 8
GPC = G // NCORES            # groups per core
COLS_IN_PC = GPC * WIN       # 512 input columns per core
COLS_OUT_PC = GPC * H        # 1024 output columns per core
NB = B // 128                # 32 batch tiles of 128 rows
CH = 8                       # batch tiles per output DMA chunk (1MB fp32)

# dtype config: f32 | f32r | f16 | bf16 for inputs/matmul, f32 | f16 | bf16 out
IN_DT = os.environ.get("KERNEL_IN_DT", "f32")
OUT_DT = os.environ.get("KERNEL_OUT_DT", "f32")
VERBOSE = os.environ.get("KERNEL_VERBOSE", "0") == "1"

_progs = {}


def _log(msg):
    if VERBOSE:
        print(f"[kernel] {msg}", flush=True)


def _np_dt(tag):
    if tag in ("f32", "f32r"):
        return np.dtype(np.float32)
    if tag == "f16":
        return np.dtype(np.float16)
    if tag == "bf16":
        import ml_dtypes

        return np.dtype(ml_dtypes.bfloat16)
    raise ValueError(tag)


def _mybir_dt(tag):
    from concourse import mybir

    return {
        "f32": mybir.dt.float32,
        "f32r": mybir.dt.float32r,
        "f16": mybir.dt.float16,
        "bf16": mybir.dt.bfloat16,
    }[tag]


def _build(in_tag, out_tag, repeat):
    from concourse import bacc, mybir, tile

    in_dt = _mybir_dt(in_tag)
    out_dt = _mybir_dt(out_tag)

    nc = bacc.Bacc(
        "TRN2", target_bir_lowering=False, debug=False, num_devices=NCORES
    )
    xT = nc.declare_dram_parameter("xT", [COLS_IN_PC, B], in_dt, isOutput=False)
    Wb = nc.declare_dram_parameter("Wb", [WIN, COLS_OUT_PC], in_dt, isOutput=False)
    out = nc.declare_dram_parameter("out", [B, COLS_OUT_PC], out_dt, isOutput=True)

    out_v = out.rearrange("(nb p) w -> nb p w", p=128)  # (NB, 128, COLS_OUT_PC)

    with tile.TileContext(nc) as tc:
        with (
            tc.tile_pool(name="w", bufs=1) as wpool,
            tc.tile_pool(name="x", bufs=2) as xpool,
            tc.tile_pool(name="o", bufs=4) as opool,
            tc.tile_pool(name="ps", bufs=8, space="PSUM") as pspool,
        ):
            wt = wpool.tile([WIN, COLS_OUT_PC], in_dt)
            nc.sync.dma_start(wt[:], Wb[:, :])
            for _rep in range(repeat):
                for g in range(GPC):
                    xt = xpool.tile([WIN, B], in_dt)
                    nc.sync.dma_start(xt[:], xT[g * WIN : (g + 1) * WIN, :])
                    for c in range(NB // CH):
                        ob = opool.tile([128, CH, H], out_dt)
                        for j in range(CH):
                            bt = c * CH + j
                            ps = pspool.tile([128, H], mybir.dt.float32)
                            nc.tensor.matmul(
                                ps[:],
                                xt[:, bt * 128 : (bt + 1) * 128],
                                wt[:, g * H : (g + 1) * H],
                                start=True,
                                stop=True,
                            )
                            if bt % 2 == 0:
                                nc.vector.tensor_scalar_max(
                                    ob[:, j, :], ps[:], 0.0
                                )
                            else:
                                nc.scalar.activation(
                                    ob[:, j, :],
                                    ps[:],
                                    mybir.ActivationFunctionType.Relu,
                                )
                        dv = out_v[
                            c * CH : (c + 1) * CH, :, g * H : (g + 1) * H
                        ].transpose([1, 0, 2])
                        nc.sync.dma_start(dv, ob[:])
    nc.compile()
    return nc


def _make_runner(nc):
    """Cached jitted shard_map runner over 8 cores (modeled on
    concourse.bass2jax.run_bass_via_pjrt, but reusable across calls:
    the jitted fn and on-device zero output buffers are kept)."""
    import jax
    from jax.experimental.shard_map import shard_map
    from jax.sharding import Mesh, NamedSharding, PartitionSpec

    from concourse import mybir
    from concourse.bass2jax import (
        _bass_exec_p,
        install_neuronx_cc_hook,
        partition_id_tensor,
    )

    install_neuronx_cc_hook()

    partition_name = (
        nc.partition_id_tensor.name if nc.partition_id_tensor else None
    )
    in_names, out_names, out_avals = [], [], []
    for alloc in nc.m.functions[0].allocations:
        if not isinstance(alloc, mybir.MemoryLocationSet):
            continue
        name = alloc.memorylocations[0].name
        if alloc.kind == "ExternalInput":
            if name != partition_name:
                in_names.append(name)
        elif alloc.kind == "ExternalOutput":
            out_names.append(name)
            out_avals.append(
                jax.core.ShapedArray(
                    tuple(alloc.tensor_shape), mybir.dt.np(alloc.dtype)
                )
            )
    n_params = len(in_names)
    all_names = in_names + out_names
    if partition_name is not None:
        all_names = all_names + [partition_name]

    def _body(*args):
        operands = list(args)
        if partition_name is not None:
            operands.append(partition_id_tensor())
        outs = _bass_exec_p.bind(
            *operands,
            out_avals=tuple(out_avals),
            in_names=tuple(all_names),
            out_names=tuple(out_names),
            lowering_input_output_aliases=(),
            sim_require_finite=True,
            sim_require_nnan=True,
            nc=nc,
        )
        return tuple(outs)

    devices = jax.devices()[:NCORES]
    mesh = Mesh(np.asarray(devices), ("core",))
    nout = len(out_names)
    fn = jax.jit(
        shard_map(
            _body,
            mesh=mesh,
            in_specs=(PartitionSpec("core"),) * (n_params + nout),
            out_specs=(PartitionSpec("core"),) * nout,
            check_rep=False,
        ),
        keep_unused=True,
    )
    sharding = NamedSharding(mesh, PartitionSpec("core"))
    zeros = [
        jax.device_put(
            np.zeros((NCORES * a.shape[0], *a.shape[1:]), a.dtype), sharding
        )
        for a in out_avals
    ]
    return {
        "fn": fn,
        "in_names": in_names,
        "out_names": out_names,
        "out_avals": out_avals,
        "sharding": sharding,
        "zeros": zeros,
    }


def get_prog(repeat=1):
    """Build (or fetch cached) compiled program + runner for the current
    dtype config and the given repeat-unroll factor."""
    key = (IN_DT, OUT_DT, repeat)
    if key not in _progs:
        t0 = time.time()
        nc = _build(IN_DT, OUT_DT, repeat)
        t1 = time.time()
        runner = _make_runner(nc)
        t2 = time.time()
        _log(
            f"built prog {key}: bass build+compile {t1 - t0:.1f}s, "
            f"runner setup {t2 - t1:.1f}s"
        )
        runner["nc"] = nc
        _progs[key] = runner
    return _progs[key]


def shard_inputs(inputs, W):
    """Host-side sharding: transpose x, extract diagonal W blocks, split
    per core, concat along axis 0 for shard_map consumption."""
    in_np = _np_dt(IN_DT)
    x = np.asarray(inputs, dtype=np.float32)
    Wf = np.asarray(W, dtype=np.float32)

    xT = np.ascontiguousarray(x.T)  # (G*WIN, B): row g*WIN+w = input col
    Wd = Wf.reshape(G, WIN, G, H)[np.arange(G), :, np.arange(G), :]  # (G,WIN,H)

    # concat over cores along axis 0 (shard_map splits axis 0 across mesh)
    xT_cat = xT.astype(in_np)  # already (NCORES*COLS_IN_PC, B) in core order
    Wb_cat = np.ascontiguousarray(
        Wd.reshape(NCORES, GPC, WIN, H)
        .transpose(0, 2, 1, 3)
        .reshape(NCORES * WIN, COLS_OUT_PC)
    ).astype(in_np)
    return {"xT": xT_cat, "Wb": Wb_cat}


def run_prog(prog, cat_inputs):
    """Run the program on 8 cores; returns concatenated outputs."""
    import jax

    args = [
        jax.device_put(cat_inputs[name], prog["sharding"])
        for name in prog["in_names"]
    ]
    outs = prog["fn"](*args, *prog["zeros"])
    jax.block_until_ready(outs)
    return outs


def kernel(inputs, W):
    prog = get_prog(repeat=1)
    cat = shard_inputs(inputs, W)
    outs = run_prog(prog, cat)
    out_cat = np.asarray(outs[prog["out_names"].index("out")])
    # (NCORES*B, COLS_OUT_PC) -> (B, NCORES*COLS_OUT_PC)
    full = np.concatenate(
        [
            out_cat[c * B : (c + 1) * B].astype(np.float32)
            for c in range(NCORES)
        ],
        axis=1,
    )
    return full


# revision 20
# speedup vs baseline: 142133.6871x; 145.0871x over previous
"""Trainium2 Bass kernel for nn_BlockDense_89730456748629.

Block-diagonal dense layer + ReLU:
    out[b, g*H+h] = relu( sum_w inputs[b, g*WIN+w] * W[g*WIN+w, g*H+h] )
with G=32 groups, WIN=128, H=256, B=4096.

Sharding: group-parallel over 8 NeuronCores — core c owns groups
[4c, 4c+4). Each core gets the matching 512 input columns of `inputs`
(pre-transposed on host so the contraction dim lies on SBUF partitions)
plus its 4 diagonal W blocks, and produces the matching 1024 output
columns. No cross-core communication.

Per-core device pipeline:
  DMA xT group-row (2MB) -> PE matmul (lhsT = xT tile [128win,128b],
  rhs = W block [128win,256h], PSUM fp32) -> ReLU fused into the
  PSUM->SBUF copy (alternating VectorE / ScalarE) -> 1MB batched DMA out.
"""

import os
import time

import numpy as np

G, WIN, H, B = 32, 128, 256, 4096
NCORES = 8
GPC = G // NCORES            # groups per core
COLS_IN_PC = GPC * WIN       # 512 input columns per core
COLS_OUT_PC = GPC * H        # 1024 output columns per core
NB = B // 128                # 32 batch tiles of 128 rows

# dtype config: f32 | f32r | f16 | bf16 for inputs/matmul, f32 | f16 | bf16 out.
# Default f16 end-to-end: measured output error is dominated by the final
# f16 rounding (~5e-4 scale-relative max) while DMA bytes (the bottleneck)
# halve vs f32.
IN_DT = os.environ.get("KERNEL_IN_DT", "f16")
OUT_DT = os.environ.get("KERNEL_OUT_DT", "f16")
# batch tiles per out-DMA chunk (2-byte out: 16 -> 4MB chunks; 4-byte: 8)
CH = int(
    os.environ.get("KERNEL_CH", "16" if OUT_DT in ("f16", "bf16") else "8")
)
VERBOSE = os.environ.get("KERNEL_VERBOSE", "0") == "1"

_progs = {}


def _log(msg):
    if VERBOSE:
        print(f"[kernel] {msg}", flush=True)


def _np_dt(tag):
    if tag in ("f32", "f32r"):
        return np.dtype(np.float32)
    if tag == "f16":
        return np.dtype(np.float16)
    if tag == "bf16":
        import ml_dtypes

        return np.dtype(ml_dtypes.bfloat16)
    raise ValueError(tag)


def _mybir_dt(tag):
    from concourse import mybir

    return {
        "f32": mybir.dt.float32,
        "f32r": mybir.dt.float32r,
        "f16": mybir.dt.float16,
        "bf16": mybir.dt.bfloat16,
    }[tag]


def _build(in_tag, out_tag, repeat, loop_n=0):
    """Build the program. `repeat` = static unroll of the whole body;
    `loop_n` > 0 additionally wraps the unrolled body in a hardware
    For_i loop with that trip count (bench-only, for timing)."""
    from concourse import bacc, mybir, tile

    # bench-only ablations: comma-set of {noin,nomm,norelu,noout}
    ablate = set(filter(None, os.environ.get("KERNEL_ABLATE", "").split(",")))
    relu_eng = os.environ.get("KERNEL_RELU", "mix")  # mix | dve | act
    psw = int(os.environ.get("KERNEL_PSW", "512"))   # psum tile width (256|512)
    layout = os.environ.get("KERNEL_LAYOUT", "bchunk")  # bchunk | group

    in_dt = _mybir_dt(in_tag)
    out_dt = _mybir_dt(out_tag)

    nc = bacc.Bacc(
        "TRN2", target_bir_lowering=False, debug=False, num_devices=NCORES
    )
    xT = nc.declare_dram_parameter("xT", [COLS_IN_PC, B], in_dt, isOutput=False)
    Wb = nc.declare_dram_parameter("Wb", [WIN, COLS_OUT_PC], in_dt, isOutput=False)
    out = nc.declare_dram_parameter("out", [B, COLS_OUT_PC], out_dt, isOutput=True)

    out_v = out.rearrange("(nb p) w -> nb p w", p=128)  # (NB, 128, COLS_OUT_PC)

    in_sz = 2 if in_tag in ("f16", "bf16") else 4
    out_sz = 2 if out_tag in ("f16", "bf16") else 4
    if layout == "bchunk":
        xbufs = 8 if in_sz == 2 else 6   # 4 resident + prefetch
        obufs = 2 if (CH >= 16 or out_sz == 4) else 3
    else:
        xbufs, obufs = 2, 4
    xbufs = int(os.environ.get("KERNEL_XBUFS", xbufs))
    obufs = int(os.environ.get("KERNEL_OBUFS", obufs))

    with tile.TileContext(nc) as tc:
        with (
            tc.tile_pool(name="w", bufs=1) as wpool,
            tc.tile_pool(name="x", bufs=xbufs) as xpool,
            tc.tile_pool(name="o", bufs=obufs) as opool,
            tc.tile_pool(name="ps", bufs=8, space="PSUM") as pspool,
        ):
            wt = wpool.tile([WIN, COLS_OUT_PC], in_dt)
            nc.sync.dma_start(wt[:], Wb[:, :])

            relu_ct = [0]

            def relu(dst, src):
                pick = relu_eng
                if pick == "mix":
                    pick = "dve" if relu_ct[0] % 2 == 0 else "act"
                relu_ct[0] += 1
                if pick == "dve":
                    nc.vector.tensor_scalar_max(dst, src, 0.0)
                else:
                    nc.scalar.activation(
                        dst, src, mybir.ActivationFunctionType.Relu
                    )

            mm_per_ps = psw // H  # matmuls per psum tile (1 or 2)

            def body_group():
                """Group-outer: xt = one group row over all B; out-DMA
                writes H-wide column strips (512B runs at f16)."""
                for _rep in range(repeat):
                    for g in range(GPC):
                        xt = xpool.tile([WIN, B], in_dt)
                        if "noin" not in ablate:
                            nc.sync.dma_start(
                                xt[:], xT[g * WIN : (g + 1) * WIN, :]
                            )
                        for c in range(NB // CH):
                            ob = opool.tile([128, CH * H], out_dt)
                            for j2 in range(CH // mm_per_ps):
                                ps = pspool.tile([128, psw], mybir.dt.float32)
                                for h in range(mm_per_ps):
                                    bt = c * CH + j2 * mm_per_ps + h
                                    if "nomm" not in ablate:
                                        nc.tensor.matmul(
                                            ps[:, h * H : (h + 1) * H],
                                            xt[:, bt * 128 : (bt + 1) * 128],
                                            wt[:, g * H : (g + 1) * H],
                                            start=True,
                                            stop=True,
                                        )
                                if "norelu" not in ablate:
                                    relu(
                                        ob[:, j2 * psw : (j2 + 1) * psw],
                                        ps[:],
                                    )
                            if "noout" not in ablate:
                                dv = out_v[
                                    c * CH : (c + 1) * CH, :, g * H : (g + 1) * H
                                ].transpose([1, 0, 2])
                                # out-DMAs ride the ACT HWDGE ring so they
                                # overlap the input DMAs on the SP ring
                                # (FIFO per ring)
                                ob3 = ob[:].rearrange("p (j h) -> p j h", h=H)
                                nc.scalar.dma_start(dv, ob3)

            def body_bchunk():
                """B-chunk-outer: all 4 group tiles resident; out-DMA
                writes full COLS_OUT_PC-wide rows (2KB runs at f16)."""
                for _rep in range(repeat):
                    xts = []
                    for g in range(GPC):
                        xt = xpool.tile([WIN, B], in_dt, tag="xt")
                        if "noin" not in ablate:
                            nc.sync.dma_start(
                                xt[:], xT[g * WIN : (g + 1) * WIN, :]
                            )
                        xts.append(xt)
                    for c in range(NB // CH):
                        ob = opool.tile([128, CH, COLS_OUT_PC], out_dt)
                        if "norelu" in ablate and "noout" not in ablate:
                            # mark ob written so Tile allocates it (bench only)
                            nc.gpsimd.memset(ob[:, 0, 0:128], 0)
                        for g in range(GPC):
                            for j2 in range(CH // mm_per_ps):
                                ps = pspool.tile([128, psw], mybir.dt.float32)
                                for h in range(mm_per_ps):
                                    bt = c * CH + j2 * mm_per_ps + h
                                    if "nomm" not in ablate:
                                        nc.tensor.matmul(
                                            ps[:, h * H : (h + 1) * H],
                                            xts[g][:, bt * 128 : (bt + 1) * 128],
                                            wt[:, g * H : (g + 1) * H],
                                            start=True,
                                            stop=True,
                                        )
                                if "norelu" not in ablate:
                                    # psum [128, (j, h)] -> ob rows j2*m+h,
                                    # group-g column strip
                                    dst = ob[
                                        :,
                                        j2 * mm_per_ps : (j2 + 1) * mm_per_ps,
                                        g * H : (g + 1) * H,
                                    ]
                                    src = ps[:].rearrange(
                                        "p (j h) -> p j h", h=H
                                    )
                                    relu(dst, src)
                        if "noout" not in ablate:
                            dv = out_v[c * CH : (c + 1) * CH, :, :].transpose(
                                [1, 0, 2]
                            )
                            nc.scalar.dma_start(dv, ob[:])

            body = body_bchunk if layout == "bchunk" else body_group

            if loop_n > 0:
                with tc.For_i(0, loop_n, 1):
                    body()
            else:
                body()
    nc.compile()
    return nc


def _make_runner(nc):
    """Cached jitted shard_map runner over 8 cores (modeled on
    concourse.bass2jax.run_bass_via_pjrt, but reusable across calls:
    the jitted fn and on-device zero output buffers are kept)."""
    import jax

    try:  # soften repeat first-call compiles across processes
        jax.config.update("jax_compilation_cache_dir", "/tmp/jax_bass_cache")
        jax.config.update("jax_persistent_cache_min_compile_time_secs", 1.0)
    except Exception:
        pass
    from jax.experimental.shard_map import shard_map
    from jax.sharding import Mesh, NamedSharding, PartitionSpec

    from concourse import mybir
    from concourse.bass2jax import (
        _bass_exec_p,
        install_neuronx_cc_hook,
        partition_id_tensor,
    )

    install_neuronx_cc_hook()

    partition_name = (
        nc.partition_id_tensor.name if nc.partition_id_tensor else None
    )
    in_names, out_names, out_avals = [], [], []
    for alloc in nc.m.functions[0].allocations:
        if not isinstance(alloc, mybir.MemoryLocationSet):
            continue
        name = alloc.memorylocations[0].name
        if alloc.kind == "ExternalInput":
            if name != partition_name:
                in_names.append(name)
        elif alloc.kind == "ExternalOutput":
            out_names.append(name)
            out_avals.append(
                jax.core.ShapedArray(
                    tuple(alloc.tensor_shape), mybir.dt.np(alloc.dtype)
                )
            )
    n_params = len(in_names)
    all_names = in_names + out_names
    if partition_name is not None:
        all_names = all_names + [partition_name]

    def _body(*args):
        operands = list(args)
        if partition_name is not None:
            operands.append(partition_id_tensor())
        outs = _bass_exec_p.bind(
            *operands,
            out_avals=tuple(out_avals),
            in_names=tuple(all_names),
            out_names=tuple(out_names),
            lowering_input_output_aliases=(),
            sim_require_finite=True,
            sim_require_nnan=True,
            nc=nc,
        )
        return tuple(outs)

    devices = jax.devices()[:NCORES]
    mesh = Mesh(np.asarray(devices), ("core",))
    nout = len(out_names)
    fn = jax.jit(
        shard_map(
            _body,
            mesh=mesh,
            in_specs=(PartitionSpec("core"),) * (n_params + nout),
            out_specs=(PartitionSpec("core"),) * nout,
            check_rep=False,
        ),
        keep_unused=True,
    )
    sharding = NamedSharding(mesh, PartitionSpec("core"))
    zeros = [
        jax.device_put(
            np.zeros((NCORES * a.shape[0], *a.shape[1:]), a.dtype), sharding
        )
        for a in out_avals
    ]
    return {
        "fn": fn,
        "in_names": in_names,
        "out_names": out_names,
        "out_avals": out_avals,
        "sharding": sharding,
        "zeros": zeros,
    }


def get_prog(repeat=1, loop_n=0):
    """Build (or fetch cached) compiled program + runner for the current
    dtype config and the given repeat-unroll / hw-loop factors."""
    key = (IN_DT, OUT_DT, repeat, loop_n)
    if key not in _progs:
        t0 = time.time()
        nc = _build(IN_DT, OUT_DT, repeat, loop_n)
        t1 = time.time()
        runner = _make_runner(nc)
        t2 = time.time()
        _log(
            f"built prog {key}: bass build+compile {t1 - t0:.1f}s, "
            f"runner setup {t2 - t1:.1f}s"
        )
        runner["nc"] = nc
        _progs[key] = runner
    return _progs[key]


def shard_inputs(inputs, W):
    """Host-side sharding: transpose x, extract diagonal W blocks, split
    per core, concat along axis 0 for shard_map consumption."""
    in_np = _np_dt(IN_DT)
    x = np.asarray(inputs, dtype=np.float32)
    Wf = np.asarray(W, dtype=np.float32)

    xT = np.ascontiguousarray(x.T)  # (G*WIN, B): row g*WIN+w = input col
    Wd = Wf.reshape(G, WIN, G, H)[np.arange(G), :, np.arange(G), :]  # (G,WIN,H)

    # concat over cores along axis 0 (shard_map splits axis 0 across mesh)
    xT_cat = xT.astype(in_np)  # already (NCORES*COLS_IN_PC, B) in core order
    Wb_cat = np.ascontiguousarray(
        Wd.reshape(NCORES, GPC, WIN, H)
        .transpose(0, 2, 1, 3)
        .reshape(NCORES * WIN, COLS_OUT_PC)
    ).astype(in_np)
    return {"xT": xT_cat, "Wb": Wb_cat}


def place_inputs(prog, cat_inputs):
    """device_put the sharded inputs once; reusable across run_prog calls."""
    import jax

    return [
        jax.device_put(cat_inputs[name], prog["sharding"])
        for name in prog["in_names"]
    ]


def run_prog(prog, cat_inputs=None, placed=None):
    """Run the program on 8 cores; returns output arrays (on device)."""
    import jax

    if placed is None:
        placed = place_inputs(prog, cat_inputs)
    outs = prog["fn"](*placed, *prog["zeros"])
    jax.block_until_ready(outs)
    return outs


def kernel(inputs, W):
    prog = get_prog(repeat=1)
    cat = shard_inputs(inputs, W)
    outs = run_prog(prog, cat)
    out_cat = np.asarray(outs[prog["out_names"].index("out")])
    # (NCORES*B, COLS_OUT_PC) -> (B, NCORES*COLS_OUT_PC)
    full = np.concatenate(
        [
            out_cat[c * B : (c + 1) * B].astype(np.float32)
            for c in range(NCORES)
        ],
        axis=1,
    )
    return full


# revision 29
# speedup vs baseline: 155163.5494x; 1.0917x over previous
"""Trainium2 Bass kernel for nn_BlockDense_89730456748629.

Block-diagonal dense layer + ReLU:
    out[b, g*H+h] = relu( sum_w inputs[b, g*WIN+w] * W[g*WIN+w, g*H+h] )
with G=32 groups, WIN=128, H=256, B=4096.

Sharding: group-parallel over 8 NeuronCores — core c owns groups
[4c, 4c+4). Each core gets the matching 512 input columns of `inputs`
(pre-transposed on host so the contraction dim lies on SBUF partitions)
plus its 4 diagonal W blocks, and produces the matching 1024 output
columns. No cross-core communication.

Per-core device pipeline:
  DMA xT group-row (2MB) -> PE matmul (lhsT = xT tile [128win,128b],
  rhs = W block [128win,256h], PSUM fp32) -> ReLU fused into the
  PSUM->SBUF copy (alternating VectorE / ScalarE) -> 1MB batched DMA out.
"""

import os
import time

import numpy as np

G, WIN, H, B = 32, 128, 256, 4096
NCORES = 8
GPC = G // NCORES            # groups per core
COLS_IN_PC = GPC * WIN       # 512 input columns per core
COLS_OUT_PC = GPC * H        # 1024 output columns per core
NB = B // 128                # 32 batch tiles of 128 rows

# dtype config: f32 | f32r | f16 | bf16 for inputs/matmul, f32 | f16 | bf16 out.
# Default f16 end-to-end: measured output error is dominated by the final
# f16 rounding (~5e-4 scale-relative max) while DMA bytes (the bottleneck)
# halve vs f32.
IN_DT = os.environ.get("KERNEL_IN_DT", "f16")
OUT_DT = os.environ.get("KERNEL_OUT_DT", "f16")
# batch tiles per out-DMA chunk (2-byte out: 16 -> 4MB chunks; 4-byte: 8)
CH = int(
    os.environ.get("KERNEL_CH", "16" if OUT_DT in ("f16", "bf16") else "8")
)
VERBOSE = os.environ.get("KERNEL_VERBOSE", "0") == "1"

_progs = {}


def _log(msg):
    if VERBOSE:
        print(f"[kernel] {msg}", flush=True)


def _np_dt(tag):
    if tag in ("f32", "f32r"):
        return np.dtype(np.float32)
    if tag == "f16":
        return np.dtype(np.float16)
    if tag == "bf16":
        import ml_dtypes

        return np.dtype(ml_dtypes.bfloat16)
    raise ValueError(tag)


def _mybir_dt(tag):
    from concourse import mybir

    return {
        "f32": mybir.dt.float32,
        "f32r": mybir.dt.float32r,
        "f16": mybir.dt.float16,
        "bf16": mybir.dt.bfloat16,
    }[tag]


def _build(in_tag, out_tag, repeat, loop_n=0):
    """Build the program. `repeat` = static unroll of the whole body;
    `loop_n` > 0 additionally wraps the unrolled body in a hardware
    For_i loop with that trip count (bench-only, for timing)."""
    from concourse import bacc, mybir, tile

    # bench-only ablations: comma-set of {noin,nomm,norelu,noout}
    ablate = set(filter(None, os.environ.get("KERNEL_ABLATE", "").split(",")))
    relu_eng = os.environ.get("KERNEL_RELU", "mix")  # mix | dve | act
    psw = int(os.environ.get("KERNEL_PSW", "512"))   # psum tile width (256|512)
    layout = os.environ.get("KERNEL_LAYOUT", "bchunk")  # bchunk | group
    # ring for input DMAs: "sync" = separate ring from out-DMAs (full
    # concurrency, HBM pays read/write turnaround), "act" = same ring as
    # out-DMAs (FIFO phases read bursts vs write bursts), "both" = alternate
    inring = os.environ.get("KERNEL_INRING", "sync")
    outring = os.environ.get("KERNEL_OUTRING", "act")  # act | both
    # phase=1: order in-DMA burst k+1 after the last out-DMA of k so HBM
    # sees alternating read/write bursts instead of mixed traffic
    phase = os.environ.get("KERNEL_PHASE", "0") == "1"

    in_dt = _mybir_dt(in_tag)
    out_dt = _mybir_dt(out_tag)

    nc = bacc.Bacc(
        "TRN2", target_bir_lowering=False, debug=False, num_devices=NCORES
    )
    xT = nc.declare_dram_parameter("xT", [COLS_IN_PC, B], in_dt, isOutput=False)
    Wb = nc.declare_dram_parameter("Wb", [WIN, COLS_OUT_PC], in_dt, isOutput=False)
    out = nc.declare_dram_parameter("out", [B, COLS_OUT_PC], out_dt, isOutput=True)

    out_v = out.rearrange("(nb p) w -> nb p w", p=128)  # (NB, 128, COLS_OUT_PC)

    in_sz = 2 if in_tag in ("f16", "bf16") else 4
    out_sz = 2 if out_tag in ("f16", "bf16") else 4
    if layout == "bchunk":
        # deep prefetch wins: 4 resident group tiles + 8 in flight ahead
        xbufs = 12 if in_sz == 2 else 6
        if out_sz == 2:
            obufs = 3 if CH >= 16 else 5
        else:
            obufs = 2
    else:
        xbufs, obufs = 2, 4
    xbufs = int(os.environ.get("KERNEL_XBUFS", xbufs))
    obufs = int(os.environ.get("KERNEL_OBUFS", obufs))

    with tile.TileContext(nc) as tc:
        with (
            tc.tile_pool(name="w", bufs=1) as wpool,
            tc.tile_pool(name="x", bufs=xbufs) as xpool,
            tc.tile_pool(name="o", bufs=obufs) as opool,
            tc.tile_pool(name="ps", bufs=8, space="PSUM") as pspool,
        ):
            wt = wpool.tile([WIN, COLS_OUT_PC], in_dt)
            nc.sync.dma_start(wt[:], Wb[:, :])

            relu_ct = [0]

            def relu(dst, src):
                pick = relu_eng
                if pick == "mix":
                    pick = "dve" if relu_ct[0] % 2 == 0 else "act"
                relu_ct[0] += 1
                if pick == "dve":
                    nc.vector.tensor_scalar_max(dst, src, 0.0)
                else:
                    nc.scalar.activation(
                        dst, src, mybir.ActivationFunctionType.Relu
                    )

            mm_per_ps = psw // H  # matmuls per psum tile (1 or 2)

            def body_group():
                """Group-outer: xt = one group row over all B; out-DMA
                writes H-wide column strips (512B runs at f16)."""
                for _rep in range(repeat):
                    for g in range(GPC):
                        xt = xpool.tile([WIN, B], in_dt)
                        if "noin" not in ablate:
                            nc.sync.dma_start(
                                xt[:], xT[g * WIN : (g + 1) * WIN, :]
                            )
                        for c in range(NB // CH):
                            ob = opool.tile([128, CH * H], out_dt)
                            for j2 in range(CH // mm_per_ps):
                                ps = pspool.tile([128, psw], mybir.dt.float32)
                                for h in range(mm_per_ps):
                                    bt = c * CH + j2 * mm_per_ps + h
                                    if "nomm" not in ablate:
                                        nc.tensor.matmul(
                                            ps[:, h * H : (h + 1) * H],
                                            xt[:, bt * 128 : (bt + 1) * 128],
                                            wt[:, g * H : (g + 1) * H],
                                            start=True,
                                            stop=True,
                                        )
                                if "norelu" not in ablate:
                                    relu(
                                        ob[:, j2 * psw : (j2 + 1) * psw],
                                        ps[:],
                                    )
                            if "noout" not in ablate:
                                dv = out_v[
                                    c * CH : (c + 1) * CH, :, g * H : (g + 1) * H
                                ].transpose([1, 0, 2])
                                # out-DMAs ride the ACT HWDGE ring so they
                                # overlap the input DMAs on the SP ring
                                # (FIFO per ring)
                                ob3 = ob[:].rearrange("p (j h) -> p j h", h=H)
                                nc.scalar.dma_start(dv, ob3)

            def body_bchunk():
                """B-chunk-outer: all 4 group tiles resident; out-DMA
                writes full COLS_OUT_PC-wide rows (2KB runs at f16)."""
                from concourse.tile import add_dep_helper

                prev_out = [None]
                for _rep in range(repeat):
                    xts = []
                    for g in range(GPC):
                        if inring == "both":
                            in_eng = nc.sync if g % 2 == 0 else nc.scalar
                        else:
                            in_eng = nc.scalar if inring == "act" else nc.sync
                        xt = xpool.tile([WIN, B], in_dt, tag="xt")
                        if "noin" not in ablate:
                            di = in_eng.dma_start(
                                xt[:], xT[g * WIN : (g + 1) * WIN, :]
                            )
                            if phase and prev_out[0] is not None:
                                add_dep_helper(
                                    prev_out[0].ins,
                                    di.ins,
                                    True,
                                    "phase reads after writes",
                                )
                        xts.append(xt)
                    for c in range(NB // CH):
                        ob = opool.tile([128, CH, COLS_OUT_PC], out_dt)
                        if "norelu" in ablate and "noout" not in ablate:
                            # mark ob written so Tile allocates it (bench only)
                            nc.gpsimd.memset(ob[:, 0, 0:128], 0)
                        for g in range(GPC):
                            for j2 in range(CH // mm_per_ps):
                                ps = pspool.tile([128, psw], mybir.dt.float32)
                                for h in range(mm_per_ps):
                                    bt = c * CH + j2 * mm_per_ps + h
                                    if "nomm" not in ablate:
                                        nc.tensor.matmul(
                                            ps[:, h * H : (h + 1) * H],
                                            xts[g][:, bt * 128 : (bt + 1) * 128],
                                            wt[:, g * H : (g + 1) * H],
                                            start=True,
                                            stop=True,
                                        )
                                if "norelu" not in ablate:
                                    # psum [128, (j, h)] -> ob rows j2*m+h,
                                    # group-g column strip
                                    dst = ob[
                                        :,
                                        j2 * mm_per_ps : (j2 + 1) * mm_per_ps,
                                        g * H : (g + 1) * H,
                                    ]
                                    src = ps[:].rearrange(
                                        "p (j h) -> p j h", h=H
                                    )
                                    relu(dst, src)
                        if "noout" not in ablate:
                            dv = out_v[c * CH : (c + 1) * CH, :, :].transpose(
                                [1, 0, 2]
                            )
                            if outring == "both":
                                out_eng = nc.scalar if c % 2 == 0 else nc.sync
                            else:
                                out_eng = nc.scalar
                            do = out_eng.dma_start(dv, ob[:])
                            prev_out[0] = do

            body = body_bchunk if layout == "bchunk" else body_group

            if loop_n > 0:
                with tc.For_i(0, loop_n, 1):
                    body()
            else:
                body()
    nc.compile()
    return nc


def _make_runner(nc):
    """Cached jitted shard_map runner over 8 cores (modeled on
    concourse.bass2jax.run_bass_via_pjrt, but reusable across calls:
    the jitted fn and on-device zero output buffers are kept)."""
    import jax

    try:  # soften repeat first-call compiles across processes
        jax.config.update("jax_compilation_cache_dir", "/tmp/jax_bass_cache")
        jax.config.update("jax_persistent_cache_min_compile_time_secs", 1.0)
    except Exception:
        pass
    from jax.experimental.shard_map import shard_map
    from jax.sharding import Mesh, NamedSharding, PartitionSpec

    from concourse import mybir
    from concourse.bass2jax import (
        _bass_exec_p,
        install_neuronx_cc_hook,
        partition_id_tensor,
    )

    install_neuronx_cc_hook()

    partition_name = (
        nc.partition_id_tensor.name if nc.partition_id_tensor else None
    )
    in_names, out_names, out_avals = [], [], []
    for alloc in nc.m.functions[0].allocations:
        if not isinstance(alloc, mybir.MemoryLocationSet):
            continue
        name = alloc.memorylocations[0].name
        if alloc.kind == "ExternalInput":
            if name != partition_name:
                in_names.append(name)
        elif alloc.kind == "ExternalOutput":
            out_names.append(name)
            out_avals.append(
                jax.core.ShapedArray(
                    tuple(alloc.tensor_shape), mybir.dt.np(alloc.dtype)
                )
            )
    n_params = len(in_names)
    all_names = in_names + out_names
    if partition_name is not None:
        all_names = all_names + [partition_name]

    def _body(*args):
        operands = list(args)
        if partition_name is not None:
            operands.append(partition_id_tensor())
        outs = _bass_exec_p.bind(
            *operands,
            out_avals=tuple(out_avals),
            in_names=tuple(all_names),
            out_names=tuple(out_names),
            lowering_input_output_aliases=(),
            sim_require_finite=True,
            sim_require_nnan=True,
            nc=nc,
        )
        return tuple(outs)

    devices = jax.devices()[:NCORES]
    mesh = Mesh(np.asarray(devices), ("core",))
    nout = len(out_names)
    fn = jax.jit(
        shard_map(
            _body,
            mesh=mesh,
            in_specs=(PartitionSpec("core"),) * (n_params + nout),
            out_specs=(PartitionSpec("core"),) * nout,
            check_rep=False,
        ),
        keep_unused=True,
    )
    sharding = NamedSharding(mesh, PartitionSpec("core"))
    zeros = [
        jax.device_put(
            np.zeros((NCORES * a.shape[0], *a.shape[1:]), a.dtype), sharding
        )
        for a in out_avals
    ]
    return {
        "fn": fn,
        "in_names": in_names,
        "out_names": out_names,
        "out_avals": out_avals,
        "sharding": sharding,
        "zeros": zeros,
    }


def get_prog(repeat=1, loop_n=0):
    """Build (or fetch cached) compiled program + runner for the current
    dtype config and the given repeat-unroll / hw-loop factors."""
    key = (IN_DT, OUT_DT, repeat, loop_n)
    if key not in _progs:
        t0 = time.time()
        nc = _build(IN_DT, OUT_DT, repeat, loop_n)
        t1 = time.time()
        runner = _make_runner(nc)
        t2 = time.time()
        _log(
            f"built prog {key}: bass build+compile {t1 - t0:.1f}s, "
            f"runner setup {t2 - t1:.1f}s"
        )
        runner["nc"] = nc
        _progs[key] = runner
    return _progs[key]


def shard_inputs(inputs, W):
    """Host-side sharding: transpose x, extract diagonal W blocks, split
    per core, concat along axis 0 for shard_map consumption."""
    in_np = _np_dt(IN_DT)
    x = np.asarray(inputs, dtype=np.float32)
    Wf = np.asarray(W, dtype=np.float32)

    xT = np.ascontiguousarray(x.T)  # (G*WIN, B): row g*WIN+w = input col
    Wd = Wf.reshape(G, WIN, G, H)[np.arange(G), :, np.arange(G), :]  # (G,WIN,H)

    # concat over cores along axis 0 (shard_map splits axis 0 across mesh)
    xT_cat = xT.astype(in_np)  # already (NCORES*COLS_IN_PC, B) in core order
    Wb_cat = np.ascontiguousarray(
        Wd.reshape(NCORES, GPC, WIN, H)
        .transpose(0, 2, 1, 3)
        .reshape(NCORES * WIN, COLS_OUT_PC)
    ).astype(in_np)
    return {"xT": xT_cat, "Wb": Wb_cat}


def place_inputs(prog, cat_inputs):
    """device_put the sharded inputs once; reusable across run_prog calls."""
    import jax

    return [
        jax.device_put(cat_inputs[name], prog["sharding"])
        for name in prog["in_names"]
    ]


def run_prog(prog, cat_inputs=None, placed=None):
    """Run the program on 8 cores; returns output arrays (on device)."""
    import jax

    if placed is None:
        placed = place_inputs(prog, cat_inputs)
    outs = prog["fn"](*placed, *prog["zeros"])
    jax.block_until_ready(outs)
    return outs


def kernel(inputs, W):
    prog = get_prog(repeat=1)
    cat = shard_inputs(inputs, W)
    outs = run_prog(prog, cat)
    out_cat = np.asarray(outs[prog["out_names"].index("out")])
    # (NCORES*B, COLS_OUT_PC) -> (B, NCORES*COLS_OUT_PC)
    full = np.concatenate(
        [
            out_cat[c * B : (c + 1) * B].astype(np.float32)
            for c in range(NCORES)
        ],
        axis=1,
    )
    return full


# revision 30
# speedup vs baseline: 156011.1973x; 1.0055x over previous
"""Trainium2 Bass kernel for nn_BlockDense_89730456748629.

Block-diagonal dense layer + ReLU:
    out[b, g*H+h] = relu( sum_w inputs[b, g*WIN+w] * W[g*WIN+w, g*H+h] )
with G=32 groups, WIN=128, H=256, B=4096.

Sharding: group-parallel over 8 NeuronCores — core c owns groups
[4c, 4c+4). Each core gets the matching 512 input columns of `inputs`
(pre-transposed on host so the contraction dim lies on SBUF partitions)
plus its 4 diagonal W blocks, and produces the matching 1024 output
columns. No cross-core communication.

Per-core device pipeline:
  DMA xT group-row (2MB) -> PE matmul (lhsT = xT tile [128win,128b],
  rhs = W block [128win,256h], PSUM fp32) -> ReLU fused into the
  PSUM->SBUF copy (alternating VectorE / ScalarE) -> 1MB batched DMA out.
"""

import os
import time

import numpy as np

G, WIN, H, B = 32, 128, 256, 4096
NCORES = 8
GPC = G // NCORES            # groups per core
COLS_IN_PC = GPC * WIN       # 512 input columns per core
COLS_OUT_PC = GPC * H        # 1024 output columns per core
NB = B // 128                # 32 batch tiles of 128 rows

# dtype config: f32 | f32r | f16 | bf16 for inputs/matmul, f32 | f16 | bf16 out.
# Default f16 end-to-end: measured output error is dominated by the final
# f16 rounding (~5e-4 scale-relative max) while DMA bytes (the bottleneck)
# halve vs f32.
IN_DT = os.environ.get("KERNEL_IN_DT", "f16")
OUT_DT = os.environ.get("KERNEL_OUT_DT", "f16")
# batch tiles per out-DMA chunk (2-byte out: 16 -> 4MB chunks; 4-byte: 8)
CH = int(
    os.environ.get("KERNEL_CH", "16" if OUT_DT in ("f16", "bf16") else "8")
)
VERBOSE = os.environ.get("KERNEL_VERBOSE", "0") == "1"

_progs = {}


def _log(msg):
    if VERBOSE:
        print(f"[kernel] {msg}", flush=True)


def _np_dt(tag):
    if tag in ("f32", "f32r"):
        return np.dtype(np.float32)
    if tag == "f16":
        return np.dtype(np.float16)
    if tag == "bf16":
        import ml_dtypes

        return np.dtype(ml_dtypes.bfloat16)
    raise ValueError(tag)


def _mybir_dt(tag):
    from concourse import mybir

    return {
        "f32": mybir.dt.float32,
        "f32r": mybir.dt.float32r,
        "f16": mybir.dt.float16,
        "bf16": mybir.dt.bfloat16,
    }[tag]


def _build(in_tag, out_tag, repeat, loop_n=0):
    """Build the program. `repeat` = static unroll of the whole body;
    `loop_n` > 0 additionally wraps the unrolled body in a hardware
    For_i loop with that trip count (bench-only, for timing)."""
    from concourse import bacc, mybir, tile

    # bench-only ablations: comma-set of {noin,nomm,norelu,noout}
    ablate = set(filter(None, os.environ.get("KERNEL_ABLATE", "").split(",")))
    relu_eng = os.environ.get("KERNEL_RELU", "mix")  # mix | dve | act
    psw = int(os.environ.get("KERNEL_PSW", "512"))   # psum tile width (256|512)
    layout = os.environ.get("KERNEL_LAYOUT", "bchunk")  # bchunk | group
    # ring for input DMAs: "sync" = separate ring from out-DMAs (full
    # concurrency, HBM pays read/write turnaround), "act" = same ring as
    # out-DMAs (FIFO phases read bursts vs write bursts), "both" = alternate
    inring = os.environ.get("KERNEL_INRING", "sync")
    outring = os.environ.get("KERNEL_OUTRING", "act")  # act | both
    # phase=1: order in-DMA burst k+1 after the last out-DMA of k so HBM
    # sees alternating read/write bursts instead of mixed traffic
    phase = os.environ.get("KERNEL_PHASE", "0") == "1"

    in_dt = _mybir_dt(in_tag)
    out_dt = _mybir_dt(out_tag)

    nc = bacc.Bacc(
        "TRN2", target_bir_lowering=False, debug=False, num_devices=NCORES
    )
    xT = nc.declare_dram_parameter("xT", [COLS_IN_PC, B], in_dt, isOutput=False)
    Wb = nc.declare_dram_parameter("Wb", [WIN, COLS_OUT_PC], in_dt, isOutput=False)
    out = nc.declare_dram_parameter("out", [B, COLS_OUT_PC], out_dt, isOutput=True)

    out_v = out.rearrange("(nb p) w -> nb p w", p=128)  # (NB, 128, COLS_OUT_PC)

    in_sz = 2 if in_tag in ("f16", "bf16") else 4
    out_sz = 2 if out_tag in ("f16", "bf16") else 4
    if layout == "bchunk":
        # deep prefetch wins: 4 resident group tiles + 8 in flight ahead
        xbufs = 12 if in_sz == 2 else 6
        if out_sz == 2:
            obufs = 3 if CH >= 16 else 5
        else:
            obufs = 2
    else:
        xbufs, obufs = 2, 4
    xbufs = int(os.environ.get("KERNEL_XBUFS", xbufs))
    obufs = int(os.environ.get("KERNEL_OBUFS", obufs))

    with tile.TileContext(nc) as tc:
        with (
            tc.tile_pool(name="w", bufs=1) as wpool,
            tc.tile_pool(name="x", bufs=xbufs) as xpool,
            tc.tile_pool(name="o", bufs=obufs) as opool,
            tc.tile_pool(name="ps", bufs=8, space="PSUM") as pspool,
        ):
            wt = wpool.tile([WIN, COLS_OUT_PC], in_dt)
            nc.sync.dma_start(wt[:], Wb[:, :])

            relu_ct = [0]

            def relu(dst, src):
                pick = relu_eng
                if pick == "mix":
                    pick = "dve" if relu_ct[0] % 2 == 0 else "act"
                relu_ct[0] += 1
                if pick == "dve":
                    nc.vector.tensor_scalar_max(dst, src, 0.0)
                else:
                    nc.scalar.activation(
                        dst, src, mybir.ActivationFunctionType.Relu
                    )

            mm_per_ps = psw // H  # matmuls per psum tile (1 or 2)

            def body_group():
                """Group-outer: xt = one group row over all B; out-DMA
                writes H-wide column strips (512B runs at f16)."""
                for _rep in range(repeat):
                    for g in range(GPC):
                        xt = xpool.tile([WIN, B], in_dt)
                        if "noin" not in ablate:
                            nc.sync.dma_start(
                                xt[:], xT[g * WIN : (g + 1) * WIN, :]
                            )
                        for c in range(NB // CH):
                            ob = opool.tile([128, CH * H], out_dt)
                            for j2 in range(CH // mm_per_ps):
                                ps = pspool.tile([128, psw], mybir.dt.float32)
                                for h in range(mm_per_ps):
                                    bt = c * CH + j2 * mm_per_ps + h
                                    if "nomm" not in ablate:
                                        nc.tensor.matmul(
                                            ps[:, h * H : (h + 1) * H],
                                            xt[:, bt * 128 : (bt + 1) * 128],
                                            wt[:, g * H : (g + 1) * H],
                                            start=True,
                                            stop=True,
                                        )
                                if "norelu" not in ablate:
                                    relu(
                                        ob[:, j2 * psw : (j2 + 1) * psw],
                                        ps[:],
                                    )
                            if "noout" not in ablate:
                                dv = out_v[
                                    c * CH : (c + 1) * CH, :, g * H : (g + 1) * H
                                ].transpose([1, 0, 2])
                                # out-DMAs ride the ACT HWDGE ring so they
                                # overlap the input DMAs on the SP ring
                                # (FIFO per ring)
                                ob3 = ob[:].rearrange("p (j h) -> p j h", h=H)
                                nc.scalar.dma_start(dv, ob3)

            def body_bchunk():
                """B-chunk-outer: all 4 group tiles resident; out-DMA
                writes full COLS_OUT_PC-wide rows (2KB runs at f16)."""
                from concourse.tile import add_dep_helper

                prev_out = [None]
                for _rep in range(repeat):
                    xts = []
                    for g in range(GPC):
                        if inring == "both":
                            in_eng = nc.sync if g % 2 == 0 else nc.scalar
                        elif inring == "gpsimd":
                            in_eng = nc.gpsimd
                        else:
                            in_eng = nc.scalar if inring == "act" else nc.sync
                        xt = xpool.tile([WIN, B], in_dt, tag="xt")
                        if "noin" not in ablate:
                            if inring == "sync2":
                                # split each group read into two halves for
                                # more descriptors in flight
                                hb = B // 2
                                for s in range(2):
                                    di = nc.sync.dma_start(
                                        xt[:, s * hb : (s + 1) * hb],
                                        xT[
                                            g * WIN : (g + 1) * WIN,
                                            s * hb : (s + 1) * hb,
                                        ],
                                    )
                            else:
                                di = in_eng.dma_start(
                                    xt[:], xT[g * WIN : (g + 1) * WIN, :]
                                )
                            if phase and prev_out[0] is not None:
                                add_dep_helper(
                                    prev_out[0].ins,
                                    di.ins,
                                    True,
                                    "phase reads after writes",
                                )
                        xts.append(xt)
                    for c in range(NB // CH):
                        ob = opool.tile([128, CH, COLS_OUT_PC], out_dt)
                        if "norelu" in ablate and "noout" not in ablate:
                            # mark ob written so Tile allocates it (bench only)
                            nc.gpsimd.memset(ob[:, 0, 0:128], 0)
                        for g in range(GPC):
                            for j2 in range(CH // mm_per_ps):
                                ps = pspool.tile([128, psw], mybir.dt.float32)
                                for h in range(mm_per_ps):
                                    bt = c * CH + j2 * mm_per_ps + h
                                    if "nomm" not in ablate:
                                        nc.tensor.matmul(
                                            ps[:, h * H : (h + 1) * H],
                                            xts[g][:, bt * 128 : (bt + 1) * 128],
                                            wt[:, g * H : (g + 1) * H],
                                            start=True,
                                            stop=True,
                                        )
                                if "norelu" not in ablate:
                                    # psum [128, (j, h)] -> ob rows j2*m+h,
                                    # group-g column strip
                                    dst = ob[
                                        :,
                                        j2 * mm_per_ps : (j2 + 1) * mm_per_ps,
                                        g * H : (g + 1) * H,
                                    ]
                                    src = ps[:].rearrange(
                                        "p (j h) -> p j h", h=H
                                    )
                                    relu(dst, src)
                        if "noout" not in ablate:
                            dv = out_v[c * CH : (c + 1) * CH, :, :].transpose(
                                [1, 0, 2]
                            )
                            if outring == "both":
                                out_eng = nc.scalar if c % 2 == 0 else nc.sync
                            else:
                                out_eng = nc.scalar
                            do = out_eng.dma_start(dv, ob[:])
                            prev_out[0] = do

            body = body_bchunk if layout == "bchunk" else body_group

            if loop_n > 0:
                with tc.For_i(0, loop_n, 1):
                    body()
            else:
                body()
    nc.compile()
    return nc


def _make_runner(nc):
    """Cached jitted shard_map runner over 8 cores (modeled on
    concourse.bass2jax.run_bass_via_pjrt, but reusable across calls:
    the jitted fn and on-device zero output buffers are kept)."""
    import jax

    try:  # soften repeat first-call compiles across processes
        jax.config.update("jax_compilation_cache_dir", "/tmp/jax_bass_cache")
        jax.config.update("jax_persistent_cache_min_compile_time_secs", 1.0)
    except Exception:
        pass
    from jax.experimental.shard_map import shard_map
    from jax.sharding import Mesh, NamedSharding, PartitionSpec

    from concourse import mybir
    from concourse.bass2jax import (
        _bass_exec_p,
        install_neuronx_cc_hook,
        partition_id_tensor,
    )

    install_neuronx_cc_hook()

    partition_name = (
        nc.partition_id_tensor.name if nc.partition_id_tensor else None
    )
    in_names, out_names, out_avals = [], [], []
    for alloc in nc.m.functions[0].allocations:
        if not isinstance(alloc, mybir.MemoryLocationSet):
            continue
        name = alloc.memorylocations[0].name
        if alloc.kind == "ExternalInput":
            if name != partition_name:
                in_names.append(name)
        elif alloc.kind == "ExternalOutput":
            out_names.append(name)
            out_avals.append(
                jax.core.ShapedArray(
                    tuple(alloc.tensor_shape), mybir.dt.np(alloc.dtype)
                )
            )
    n_params = len(in_names)
    all_names = in_names + out_names
    if partition_name is not None:
        all_names = all_names + [partition_name]

    def _body(*args):
        operands = list(args)
        if partition_name is not None:
            operands.append(partition_id_tensor())
        outs = _bass_exec_p.bind(
            *operands,
            out_avals=tuple(out_avals),
            in_names=tuple(all_names),
            out_names=tuple(out_names),
            lowering_input_output_aliases=(),
            sim_require_finite=True,
            sim_require_nnan=True,
            nc=nc,
        )
        return tuple(outs)

    devices = jax.devices()[:NCORES]
    mesh = Mesh(np.asarray(devices), ("core",))
    nout = len(out_names)
    fn = jax.jit(
        shard_map(
            _body,
            mesh=mesh,
            in_specs=(PartitionSpec("core"),) * (n_params + nout),
            out_specs=(PartitionSpec("core"),) * nout,
            check_rep=False,
        ),
        keep_unused=True,
    )
    sharding = NamedSharding(mesh, PartitionSpec("core"))
    zeros = [
        jax.device_put(
            np.zeros((NCORES * a.shape[0], *a.shape[1:]), a.dtype), sharding
        )
        for a in out_avals
    ]
    return {
        "fn": fn,
        "in_names": in_names,
        "out_names": out_names,
        "out_avals": out_avals,
        "sharding": sharding,
        "zeros": zeros,
    }


def get_prog(repeat=1, loop_n=0):
    """Build (or fetch cached) compiled program + runner for the current
    dtype config and the given repeat-unroll / hw-loop factors."""
    key = (IN_DT, OUT_DT, repeat, loop_n)
    if key not in _progs:
        t0 = time.time()
        nc = _build(IN_DT, OUT_DT, repeat, loop_n)
        t1 = time.time()
        runner = _make_runner(nc)
        t2 = time.time()
        _log(
            f"built prog {key}: bass build+compile {t1 - t0:.1f}s, "
            f"runner setup {t2 - t1:.1f}s"
        )
        runner["nc"] = nc
        _progs[key] = runner
    return _progs[key]


def shard_inputs(inputs, W):
    """Host-side sharding: transpose x, extract diagonal W blocks, split
    per core, concat along axis 0 for shard_map consumption."""
    in_np = _np_dt(IN_DT)
    x = np.asarray(inputs, dtype=np.float32)
    Wf = np.asarray(W, dtype=np.float32)

    xT = np.ascontiguousarray(x.T)  # (G*WIN, B): row g*WIN+w = input col
    Wd = Wf.reshape(G, WIN, G, H)[np.arange(G), :, np.arange(G), :]  # (G,WIN,H)

    # concat over cores along axis 0 (shard_map splits axis 0 across mesh)
    xT_cat = xT.astype(in_np)  # already (NCORES*COLS_IN_PC, B) in core order
    Wb_cat = np.ascontiguousarray(
        Wd.reshape(NCORES, GPC, WIN, H)
        .transpose(0, 2, 1, 3)
        .reshape(NCORES * WIN, COLS_OUT_PC)
    ).astype(in_np)
    return {"xT": xT_cat, "Wb": Wb_cat}


def place_inputs(prog, cat_inputs):
    """device_put the sharded inputs once; reusable across run_prog calls."""
    import jax

    return [
        jax.device_put(cat_inputs[name], prog["sharding"])
        for name in prog["in_names"]
    ]


def run_prog(prog, cat_inputs=None, placed=None):
    """Run the program on 8 cores; returns output arrays (on device)."""
    import jax

    if placed is None:
        placed = place_inputs(prog, cat_inputs)
    outs = prog["fn"](*placed, *prog["zeros"])
    jax.block_until_ready(outs)
    return outs


def kernel(inputs, W):
    prog = get_prog(repeat=1)
    cat = shard_inputs(inputs, W)
    outs = run_prog(prog, cat)
    out_cat = np.asarray(outs[prog["out_names"].index("out")])
    # (NCORES*B, COLS_OUT_PC) -> (B, NCORES*COLS_OUT_PC)
    full = np.concatenate(
        [
            out_cat[c * B : (c + 1) * B].astype(np.float32)
            for c in range(NCORES)
        ],
        axis=1,
    )
    return full
